# revision 18
# baseline (speedup 1.0000x reference)
"""Additive attention (B=4, Q=256, KV=1024, H=128, VS=256) on 8 Trainium2 cores.

Sharding: each core processes 32 query rows of every batch (4 groups of 32
row-slots).  Per batch, only ceil(valid_len/128) KV chunks of 128 are computed;
masked columns beyond that contribute exactly 0 to the softmax, so skipping
them is exact.  No collectives are needed.

Per-core dataflow:
  DVE   : sums[h, kv] = kp[h, kv] + qp[h, s]        (tensor_scalar add)
  ACT   : tanh in place over 8-row batches           (the throughput bottleneck)
  PE    : score rows via one-hot-wv matmuls accumulated into PSUM partitions,
          mask added with one K=4 matmul (ind ⊗ mask),
          probs transposes, final attn @ V in 32-column strips per group
  DVE   : softmax max / sum / reciprocal, final scale
"""
import math
import os
import sys

import numpy as np

for _p in ("/opt/trn_rl_repo", "/root/.axon_site/_ro/trn_rl_repo"):
    if os.path.isdir(_p):
        if _p not in sys.path:
            sys.path.insert(0, _p)
        break

B, Q, KV, QS, KS, H, VS = 4, 256, 1024, 128, 128, 128, 256
P = 128
N_CORES = 8
GROUP_ROWS = 32          # rows per (core, batch)
SUB = 8                  # rows per tanh batch

PROFILE = False          # set by test.py; enables NTFF tracing
LAST_RESULTS = None
SIMULATE = False         # set by test.py; run CoreSim instead of hardware
LAST_EXEC_NS = None

_prog_cache = {}


def _build_program(cfg):
    """cfg: (ncfg, l0flags): per-group KV chunk counts (sorted desc) and
    per-group valid_len==0 flags. Returns nc."""
    ncfg, l0flags = cfg
    import contextlib

    import concourse.bacc as bacc
    import concourse.mybir as mybir
    import concourse.tile as tile
    from concourse.tile_rust import add_dep_helper

    f32 = mybir.dt.float32
    W = [c * P for c in ncfg]           # per-group computed KV width
    Wmax = W[0]
    nc = bacc.Bacc("TRN2", target_bir_lowering=False, debug=False,
                   enable_asserts=True, num_devices=N_CORES)

    blob_d = nc.dram_tensor("blob", [P, 4 * P], f32,
                            kind="ExternalInput").ap()
    wvdb_d = nc.dram_tensor("wvdb", [P, 2 * (2 * P - 1)], mybir.dt.float32r,
                            kind="ExternalInput").ap()
    kT_d = nc.dram_tensor("kT", [P, B * KV], f32, kind="ExternalInput").ap()
    V_d = nc.dram_tensor("V", [B, KV, VS], f32, kind="ExternalInput").ap()
    ind_d = nc.dram_tensor("ind", [B, P], mybir.dt.float32r,
                           kind="ExternalInput").ap()
    wvd0_d = nc.dram_tensor("wvd0", [P, 2 * P - 1], mybir.dt.float32r,
                            kind="ExternalInput").ap()
    mask_d = nc.dram_tensor("mask", [B, Wmax], mybir.dt.float32r,
                            kind="ExternalInput").ap()
    out_d = nc.dram_tensor("out", [P, VS], f32, kind="ExternalOutput").ap()

    with tile.TileContext(nc) as tc, contextlib.ExitStack() as ctx:
        const = ctx.enter_context(tc.tile_pool(name="const", bufs=1))
        ktp = ctx.enter_context(tc.tile_pool(name="ktp", bufs=2))
        feats_pool = ctx.enter_context(tc.tile_pool(name="featsp", bufs=3))
        small = ctx.enter_context(tc.tile_pool(name="small", bufs=1))
        psum = ctx.enter_context(tc.tile_pool(name="psum", bufs=1, space="PSUM"))
        psum2 = ctx.enter_context(tc.tile_pool(name="psum2", bufs=2, space="PSUM"))

        # ---- constant loads: one blob DMA on the critical Sync queue;
        # small leftovers go through the idle GpSimd issue queue ----
        f32r = mybir.dt.float32r
        blob = const.tile([P, 4 * P], f32)
        nc.sync.dma_start(blob[:], blob_d[:])
        qt_sb = blob[:, 0:P]
        wq_sb = blob[:, P:2 * P]
        wk_sb = blob[:, 2 * P:3 * P]
        ident_sb = blob[:, 3 * P:4 * P]
        wvdb = const.tile([P, 2 * (2 * P - 1)], f32r)
        nc.gpsimd.dma_start(wvdb[:], wvdb_d[:])
        wvd_hi = wvdb[:, 0:2 * P - 1]
        wvd_lo = wvdb[:, 2 * P - 1:]
        ind_sb = const.tile([B, P], f32r)
        nc.gpsimd.dma_start(ind_sb[:], ind_d[:])
        mask_sb = const.tile([B, Wmax], f32r)
        nc.gpsimd.dma_start(mask_sb[:], mask_d[:])

        # ---- projections ----
        qp_ps = psum2.tile([P, P], f32, tag="proj")
        nc.tensor.matmul(qp_ps[:], wq_sb[:], qt_sb[:], start=True, stop=True)
        qp_sb = const.tile([P, P], f32)
        nc.vector.tensor_copy(qp_sb[:], qp_ps[:])

        kp_sb = const.tile([P, B * KV], f32)
        last_kp_copy = None
        for g in range(B):
            kt_t = ktp.tile([P, KV], f32, tag="kt", name=f"kt_{g}")
            nc.sync.dma_start(kt_t[:, :W[g]], kT_d[:, g * KV: g * KV + W[g]])
            for j in range(0, W[g], 512):
                n = min(512, W[g] - j)
                kp_ps = psum2.tile([P, 512], f32, tag="proj", name=f"kp_ps_{g}_{j}")
                nc.tensor.matmul(kp_ps[:, :n], wk_sb[:], kt_t[:, j: j + n],
                                 start=True, stop=True)
                last_kp_copy = nc.vector.tensor_copy(
                    kp_sb[:, g * KV + j: g * KV + j + n], kp_ps[:, :n])

        # wvd windows: wvd_hi[:, 127-s : 255-s] has wv_hi exactly at window
        # column s, zeros elsewhere (one-hot); wvd_lo carries the residual
        if any(l0flags):
            wvd0_t = const.tile([P, 2 * P - 1], f32r)
            nc.gpsimd.dma_start(wvd0_t[:], wvd0_d[:])
            wvd0 = wvd0_t[:]

        # ---- V tiles: DMAs issued from the Scalar queue inside the main
        # loop so they don't compete with kT for HBM bandwidth in the head --
        v_chunks = [(g, c) for g in range(B) for c in range(ncfg[g])]
        vts = {}
        for g, c in v_chunks:
            vts[(g, c)] = const.tile([P, VS], f32, name=f"v_{g}_{c}")
            vdma = nc.sync.dma_start(vts[(g, c)][:],
                                     V_d[g, c * P:(c + 1) * P, :])
            add_dep_helper(vdma.ins, last_kp_copy.ins,
                           reason="V loads wait for kp so kT wins head HBM bw")

        # ---- main loop: scores ----
        scores_ps = psum.tile([P, Wmax], f32)
        # mask first (start=True clears each bank); matvecs accumulate onto it
        for c0 in range(0, Wmax, 512):
            c1 = min(c0 + 512, Wmax)
            nc.tensor.matmul(scores_ps[:, c0:c1], ind_sb[:], mask_sb[:, c0:c1],
                             start=True, stop=False, skip_group_check=True)
        for g in range(B):
            wg = W[g]
            for sb in range(GROUP_ROWS // SUB):
                feats = feats_pool.tile([P, SUB * wg], f32r, tag="feats",
                                        name=f"feats_{g}_{sb}")
                for j in range(SUB):
                    s = GROUP_ROWS * g + SUB * sb + j
                    nc.vector.tensor_scalar_add(
                        feats[:, j * wg:(j + 1) * wg],
                        kp_sb[:, g * KV: g * KV + wg],
                        qp_sb[:, s: s + 1])
                nc.scalar.activation(feats[:], feats[:],
                                     mybir.ActivationFunctionType.Tanh)
                wsrcs = [wvd0] if l0flags[g] else [wvd_hi, wvd_lo]
                for j in range(SUB):
                    s = GROUP_ROWS * g + SUB * sb + j
                    last = (g == B - 1 and sb == GROUP_ROWS // SUB - 1
                            and j == SUB - 1)
                    for wsrc in wsrcs:
                        for c0 in range(0, wg, 512):
                            c1 = min(c0 + 512, wg)
                            nc.tensor.matmul(
                                scores_ps[:, c0:c1],
                                wsrc[:, P - 1 - s: 2 * P - 1 - s],
                                feats[:, j * wg + c0: j * wg + c1],
                                start=False,
                                stop=(last and wsrc is wsrcs[-1]
                                      and c0 + 512 >= wg),
                                skip_group_check=True)

        # ---- softmax ----
        nrowmax = small.tile([P, 1], f32)
        nc.vector.reduce_max(nrowmax[:], scores_ps[:, :Wmax],
                             axis=mybir.AxisListType.X, negate=True)
        probs = small.tile([P, Wmax], f32)
        n_ec = (Wmax + 255) // 256
        psums = small.tile([P, n_ec], f32)
        for e in range(n_ec):
            e0, e1 = e * 256, min((e + 1) * 256, Wmax)
            nc.scalar.activation(probs[:, e0:e1], scores_ps[:, e0:e1],
                                 mybir.ActivationFunctionType.Exp,
                                 bias=nrowmax[:, 0:1], scale=1.0,
                                 accum_out=psums[:, e:e + 1])
        rowsum = small.tile([P, 1], f32)
        nc.vector.reduce_sum(rowsum[:], psums[:], axis=mybir.AxisListType.X)
        rinv = small.tile([P, 1], f32)
        nc.vector.reciprocal(rinv[:], rowsum[:])

        out_ps = psum.tile([P, VS], f32, name="out_ps")
        for c in range(ncfg[0]):
            pt_ps = psum2.tile([P, P], f32, tag="pt", name=f"pt_ps{c}")
            nc.tensor.transpose(pt_ps[:], probs[:, c * P:(c + 1) * P], ident_sb[:])
            pt_sb = small.tile([P, P], f32, name=f"pt_sb{c}")
            nc.vector.tensor_copy(pt_sb[:], pt_ps[:])
            for g in range(B):
                if c < ncfg[g]:
                    nc.tensor.matmul(
                        out_ps[GROUP_ROWS * g: GROUP_ROWS * (g + 1), :],
                        pt_sb[:, GROUP_ROWS * g: GROUP_ROWS * (g + 1)],
                        vts[(g, c)][:],
                        start=(c == 0), stop=(c == ncfg[g] - 1),
                        tile_position=(0, GROUP_ROWS * g),
                        skip_group_check=True)

        out_sb = small.tile([P, VS], f32)
        nc.vector.tensor_scalar_mul(out_sb[:], out_ps[:], rinv[:, 0:1])
        nc.sync.dma_start(out_d[:], out_sb[:])

    nc.compile()
    return nc


def _get_program(ncfg):
    if ncfg not in _prog_cache:
        _prog_cache[ncfg] = _build_program(ncfg)
    return _prog_cache[ncfg]


def kernel(queries, keys, values, valid_lens, Wq, Wk, wv):
    global LAST_EXEC_NS
    queries = np.ascontiguousarray(np.asarray(queries), dtype=np.float32)
    keys = np.ascontiguousarray(np.asarray(keys), dtype=np.float32)
    values = np.ascontiguousarray(np.asarray(values), dtype=np.float32)
    Wq = np.ascontiguousarray(np.asarray(Wq), dtype=np.float32)
    Wk = np.ascontiguousarray(np.asarray(Wk), dtype=np.float32)
    wv = np.ascontiguousarray(np.asarray(wv), dtype=np.float32)
    vl = [int(x) for x in np.asarray(valid_lens)]

    nc_b = [min(8, max(1, math.ceil(L / P))) if L > 0 else 8 for L in vl]
    # fp32r matmuls need moving dim >= 256: avoid widths of 128 or 512+128
    nc_b = [{1: 2, 5: 6}.get(c, c) for c in nc_b]
    order = sorted(range(B), key=lambda b: (-nc_b[b], b))
    ncfg = tuple(nc_b[b] for b in order)
    l0flags = tuple(vl[order[g]] == 0 for g in range(B))
    Wmax = ncfg[0] * P

    nc = _get_program((ncfg, l0flags))

    kT = np.concatenate([keys[order[g]].T for g in range(B)], axis=1)
    kT = np.ascontiguousarray(kT)                        # [128, 4096]
    Vm = np.ascontiguousarray(np.stack([values[order[g]] for g in range(B)]))
    ind = np.zeros((B, P), np.float32)
    for g in range(B):
        ind[g, GROUP_ROWS * g: GROUP_ROWS * (g + 1)] = 1.0
    mask = np.full((B, Wmax), -1e6, np.float32)
    for g in range(B):
        L = vl[order[g]]
        if L > 0:
            mask[g, :min(L, Wmax)] = 0.0
        else:
            mask[g, :] = 0.0
    ident = np.eye(P, dtype=np.float32)

    wv_hi = (wv.view(np.uint32) & np.uint32(0xFFFF0000)).view(np.float32)
    DW = 2 * P - 1
    blob = np.zeros((P, 4 * P), np.float32)
    blob[:, P:2 * P] = Wq
    blob[:, 2 * P:3 * P] = Wk
    blob[:, 3 * P:4 * P] = ident
    wvdb = np.zeros((P, 2 * DW), np.float32)
    wvdb[:, P - 1] = wv_hi
    wvdb[:, DW + P - 1] = wv - wv_hi
    wvd0 = np.zeros((P, DW), np.float32)
    shared = {"kT": kT, "V": Vm, "ind": ind, "mask": mask, "wvd0": wvd0,
              "wvdb": wvdb}
    in_maps = []
    for c in range(N_CORES):
        qT = np.concatenate(
            [queries[order[g], c * GROUP_ROWS:(c + 1) * GROUP_ROWS, :].T
             for g in range(B)], axis=1)
        bl = blob.copy()
        bl[:, 0:P] = qT
        m = dict(shared)
        m["blob"] = bl
        in_maps.append(m)

    if SIMULATE:
        from concourse.bass_interp import CoreSim
        outs = []
        for c in range(N_CORES):
            sim = CoreSim(nc, trace=False)
            for name, v in in_maps[c].items():
                sim.tensor(name)[:] = v
            sim.simulate(check_with_hw=False)
            outs.append(sim.tensor("out").copy())
    else:
        from concourse import bass_utils
        kw = {}
        if PROFILE:
            kw = {"trace": True}
        res = bass_utils.run_bass_kernel_spmd(nc, in_maps, list(range(N_CORES)),
                                              **kw)
        if PROFILE:
            LAST_EXEC_NS = res.exec_time_ns
            global LAST_RESULTS
            LAST_RESULTS = res
        outs = [res.results[c]["out"] for c in range(N_CORES)]

    out = np.zeros((B, Q, VS), np.float32)
    for c in range(N_CORES):
        for g in range(B):
            out[order[g], c * GROUP_ROWS:(c + 1) * GROUP_ROWS, :] = \
                outs[c][GROUP_ROWS * g: GROUP_ROWS * (g + 1), :]
    return out


# revision 19
# speedup vs baseline: 1.0819x; 1.0819x over previous
"""Additive attention (B=4, Q=256, KV=1024, H=128, VS=256) on 8 Trainium2 cores.

Sharding: each core processes 32 query rows of every batch (4 groups of 32
row-slots).  Per batch, only ceil(valid_len/128) KV chunks of 128 are computed;
masked columns beyond that contribute exactly 0 to the softmax, so skipping
them is exact.  No collectives are needed.

Per-core dataflow:
  DVE   : sums[h, kv] = kp[h, kv] + qp[h, s]        (tensor_scalar add)
  ACT   : tanh in place over 8-row batches           (the throughput bottleneck)
  PE    : score rows via one-hot-wv matmuls accumulated into PSUM partitions,
          mask added with one K=4 matmul (ind ⊗ mask),
          probs transposes, final attn @ V in 32-column strips per group
  DVE   : softmax max / sum / reciprocal, final scale
"""
import math
import os
import sys

import numpy as np

for _p in ("/opt/trn_rl_repo", "/root/.axon_site/_ro/trn_rl_repo"):
    if os.path.isdir(_p):
        if _p not in sys.path:
            sys.path.insert(0, _p)
        break

B, Q, KV, QS, KS, H, VS = 4, 256, 1024, 128, 128, 128, 256
P = 128
N_CORES = 8
GROUP_ROWS = 32          # rows per (core, batch)
SUB = 8                  # rows per tanh batch

PROFILE = False          # set by test.py; enables NTFF tracing
LO_PASS = True           # wv hi/lo split second matvec pass (precision)
LAST_RESULTS = None
SIMULATE = False         # set by test.py; run CoreSim instead of hardware
LAST_EXEC_NS = None

_prog_cache = {}


def _build_program(cfg):
    """cfg: (ncfg, l0flags): per-group KV chunk counts (sorted desc) and
    per-group valid_len==0 flags. Returns nc."""
    ncfg, l0flags, _lo = cfg
    import contextlib

    import concourse.bacc as bacc
    import concourse.mybir as mybir
    import concourse.tile as tile
    from concourse.tile_rust import add_dep_helper

    f32 = mybir.dt.float32
    W = [c * P for c in ncfg]           # per-group computed KV width
    Wmax = W[0]
    nc = bacc.Bacc("TRN2", target_bir_lowering=False, debug=False,
                   enable_asserts=True, num_devices=N_CORES)

    blob_d = nc.dram_tensor("blob", [P, 4 * P], f32,
                            kind="ExternalInput").ap()
    wvdb_d = nc.dram_tensor("wvdb", [P, 2 * (2 * P - 1)], mybir.dt.float32r,
                            kind="ExternalInput").ap()
    kT_d = nc.dram_tensor("kT", [P, B * KV], f32, kind="ExternalInput").ap()
    V_d = nc.dram_tensor("V", [B, KV, VS], f32, kind="ExternalInput").ap()
    ind_d = nc.dram_tensor("ind", [B, P], mybir.dt.float32r,
                           kind="ExternalInput").ap()
    wvd0_d = nc.dram_tensor("wvd0", [P, 2 * P - 1], mybir.dt.float32r,
                            kind="ExternalInput").ap()
    mask_d = nc.dram_tensor("mask", [B, Wmax], mybir.dt.float32r,
                            kind="ExternalInput").ap()
    out_d = nc.dram_tensor("out", [P, VS], f32, kind="ExternalOutput").ap()

    with tile.TileContext(nc) as tc, contextlib.ExitStack() as ctx:
        const = ctx.enter_context(tc.tile_pool(name="const", bufs=1))
        ktp = ctx.enter_context(tc.tile_pool(name="ktp", bufs=2))
        feats_pool = ctx.enter_context(tc.tile_pool(name="featsp", bufs=3))
        small = ctx.enter_context(tc.tile_pool(name="small", bufs=1))
        psum = ctx.enter_context(tc.tile_pool(name="psum", bufs=1, space="PSUM"))
        psum2 = ctx.enter_context(tc.tile_pool(name="psum2", bufs=2, space="PSUM"))

        # ---- constant loads: one blob DMA on the critical Sync queue;
        # small leftovers go through the idle GpSimd issue queue ----
        f32r = mybir.dt.float32r
        blob = const.tile([P, 4 * P], f32)
        nc.sync.dma_start(blob[:], blob_d[:])
        qt_sb = blob[:, 0:P]
        wq_sb = blob[:, P:2 * P]
        wk_sb = blob[:, 2 * P:3 * P]
        ident_sb = blob[:, 3 * P:4 * P]
        wvdb = const.tile([P, 2 * (2 * P - 1)], f32r)
        nc.gpsimd.dma_start(wvdb[:], wvdb_d[:])
        wvd_hi = wvdb[:, 0:2 * P - 1]
        wvd_lo = wvdb[:, 2 * P - 1:]
        ind_sb = const.tile([B, P], f32r)
        nc.gpsimd.dma_start(ind_sb[:], ind_d[:])
        mask_sb = const.tile([B, Wmax], f32r)
        nc.gpsimd.dma_start(mask_sb[:], mask_d[:])

        # ---- projections ----
        qp_ps = psum2.tile([P, P], f32, tag="proj")
        nc.tensor.matmul(qp_ps[:], wq_sb[:], qt_sb[:], start=True, stop=True)
        qp_sb = const.tile([P, P], f32)
        nc.vector.tensor_copy(qp_sb[:], qp_ps[:])

        kp_sb = const.tile([P, B * KV], f32)
        last_kp_copy = None
        for g in range(B):
            kt_t = ktp.tile([P, KV], f32, tag="kt", name=f"kt_{g}")
            nc.sync.dma_start(kt_t[:, :W[g]], kT_d[:, g * KV: g * KV + W[g]])
            for j in range(0, W[g], 512):
                n = min(512, W[g] - j)
                kp_ps = psum2.tile([P, 512], f32, tag="proj", name=f"kp_ps_{g}_{j}")
                nc.tensor.matmul(kp_ps[:, :n], wk_sb[:], kt_t[:, j: j + n],
                                 start=True, stop=True)
                last_kp_copy = nc.vector.tensor_copy(
                    kp_sb[:, g * KV + j: g * KV + j + n], kp_ps[:, :n])

        # wvd windows: wvd_hi[:, 127-s : 255-s] has wv_hi exactly at window
        # column s, zeros elsewhere (one-hot); wvd_lo carries the residual
        if any(l0flags):
            wvd0_t = const.tile([P, 2 * P - 1], f32r)
            nc.gpsimd.dma_start(wvd0_t[:], wvd0_d[:])
            wvd0 = wvd0_t[:]

        # ---- V tiles: DMAs issued from the Scalar queue inside the main
        # loop so they don't compete with kT for HBM bandwidth in the head --
        v_chunks = [(g, c) for g in range(B) for c in range(ncfg[g])]
        vts = {}
        for g, c in v_chunks:
            vts[(g, c)] = const.tile([P, VS], f32, name=f"v_{g}_{c}")
            vdma = nc.sync.dma_start(vts[(g, c)][:],
                                     V_d[g, c * P:(c + 1) * P, :])
            add_dep_helper(vdma.ins, last_kp_copy.ins,
                           reason="V loads wait for kp so kT wins head HBM bw")

        # ---- main loop: scores ----
        scores_ps = psum.tile([P, Wmax], f32)
        # mask first (start=True clears each bank); matvecs accumulate onto it
        for c0 in range(0, Wmax, 512):
            c1 = min(c0 + 512, Wmax)
            nc.tensor.matmul(scores_ps[:, c0:c1], ind_sb[:], mask_sb[:, c0:c1],
                             start=True, stop=False, skip_group_check=True)
        for g in range(B):
            wg = W[g]
            for sb in range(GROUP_ROWS // SUB):
                feats = feats_pool.tile([P, SUB * wg], f32r, tag="feats",
                                        name=f"feats_{g}_{sb}")
                for j in range(SUB):
                    s = GROUP_ROWS * g + SUB * sb + j
                    nc.vector.tensor_scalar_add(
                        feats[:, j * wg:(j + 1) * wg],
                        kp_sb[:, g * KV: g * KV + wg],
                        qp_sb[:, s: s + 1])
                nc.scalar.activation(feats[:], feats[:],
                                     mybir.ActivationFunctionType.Tanh)
                wsrcs = [wvd0] if l0flags[g] else ([wvd_hi, wvd_lo]
                         if _lo else [wvd_hi])
                for j in range(SUB):
                    s = GROUP_ROWS * g + SUB * sb + j
                    last = (g == B - 1 and sb == GROUP_ROWS // SUB - 1
                            and j == SUB - 1)
                    for wsrc in wsrcs:
                        for c0 in range(0, wg, 512):
                            c1 = min(c0 + 512, wg)
                            nc.tensor.matmul(
                                scores_ps[:, c0:c1],
                                wsrc[:, P - 1 - s: 2 * P - 1 - s],
                                feats[:, j * wg + c0: j * wg + c1],
                                start=False,
                                stop=(last and wsrc is wsrcs[-1]
                                      and c0 + 512 >= wg),
                                skip_group_check=True)

        # ---- softmax ----
        nrowmax = small.tile([P, 1], f32)
        nc.vector.reduce_max(nrowmax[:], scores_ps[:, :Wmax],
                             axis=mybir.AxisListType.X, negate=True)
        probs = small.tile([P, Wmax], f32)
        n_ec = (Wmax + 255) // 256
        psums = small.tile([P, n_ec], f32)
        for e in range(n_ec):
            e0, e1 = e * 256, min((e + 1) * 256, Wmax)
            nc.scalar.activation(probs[:, e0:e1], scores_ps[:, e0:e1],
                                 mybir.ActivationFunctionType.Exp,
                                 bias=nrowmax[:, 0:1], scale=1.0,
                                 accum_out=psums[:, e:e + 1])
        rowsum = small.tile([P, 1], f32)
        nc.vector.reduce_sum(rowsum[:], psums[:], axis=mybir.AxisListType.X)
        rinv = small.tile([P, 1], f32)
        nc.vector.reciprocal(rinv[:], rowsum[:])

        out_ps = psum.tile([P, VS], f32, name="out_ps")
        for c in range(ncfg[0]):
            pt_ps = psum2.tile([P, P], f32, tag="pt", name=f"pt_ps{c}")
            nc.tensor.transpose(pt_ps[:], probs[:, c * P:(c + 1) * P], ident_sb[:])
            pt_sb = small.tile([P, P], f32, name=f"pt_sb{c}")
            nc.vector.tensor_copy(pt_sb[:], pt_ps[:])
            for g in range(B):
                if c < ncfg[g]:
                    nc.tensor.matmul(
                        out_ps[GROUP_ROWS * g: GROUP_ROWS * (g + 1), :],
                        pt_sb[:, GROUP_ROWS * g: GROUP_ROWS * (g + 1)],
                        vts[(g, c)][:],
                        start=(c == 0), stop=(c == ncfg[g] - 1),
                        tile_position=(0, GROUP_ROWS * g),
                        skip_group_check=True)

        out_sb = small.tile([P, VS], f32)
        nc.vector.tensor_scalar_mul(out_sb[:], out_ps[:], rinv[:, 0:1])
        nc.sync.dma_start(out_d[:], out_sb[:])

    nc.compile()
    return nc


def _get_program(ncfg):
    if ncfg not in _prog_cache:
        _prog_cache[ncfg] = _build_program(ncfg)
    return _prog_cache[ncfg]


def kernel(queries, keys, values, valid_lens, Wq, Wk, wv):
    global LAST_EXEC_NS
    queries = np.ascontiguousarray(np.asarray(queries), dtype=np.float32)
    keys = np.ascontiguousarray(np.asarray(keys), dtype=np.float32)
    values = np.ascontiguousarray(np.asarray(values), dtype=np.float32)
    Wq = np.ascontiguousarray(np.asarray(Wq), dtype=np.float32)
    Wk = np.ascontiguousarray(np.asarray(Wk), dtype=np.float32)
    wv = np.ascontiguousarray(np.asarray(wv), dtype=np.float32)
    vl = [int(x) for x in np.asarray(valid_lens)]

    nc_b = [min(8, max(1, math.ceil(L / P))) if L > 0 else 8 for L in vl]
    # fp32r matmuls need moving dim >= 256: avoid widths of 128 or 512+128
    nc_b = [{1: 2, 5: 6}.get(c, c) for c in nc_b]
    order = sorted(range(B), key=lambda b: (-nc_b[b], b))
    ncfg = tuple(nc_b[b] for b in order)
    l0flags = tuple(vl[order[g]] == 0 for g in range(B))
    Wmax = ncfg[0] * P

    nc = _get_program((ncfg, l0flags, LO_PASS))

    kT = np.concatenate([keys[order[g]].T for g in range(B)], axis=1)
    kT = np.ascontiguousarray(kT)                        # [128, 4096]
    Vm = np.ascontiguousarray(np.stack([values[order[g]] for g in range(B)]))
    ind = np.zeros((B, P), np.float32)
    for g in range(B):
        ind[g, GROUP_ROWS * g: GROUP_ROWS * (g + 1)] = 1.0
    mask = np.full((B, Wmax), -1e6, np.float32)
    for g in range(B):
        L = vl[order[g]]
        if L > 0:
            mask[g, :min(L, Wmax)] = 0.0
        else:
            mask[g, :] = 0.0
    ident = np.eye(P, dtype=np.float32)

    wv_hi = (wv.view(np.uint32) & np.uint32(0xFFFF0000)).view(np.float32)
    DW = 2 * P - 1
    blob = np.zeros((P, 4 * P), np.float32)
    blob[:, P:2 * P] = Wq
    blob[:, 2 * P:3 * P] = Wk
    blob[:, 3 * P:4 * P] = ident
    wvdb = np.zeros((P, 2 * DW), np.float32)
    wvdb[:, P - 1] = wv_hi if LO_PASS else wv
    wvdb[:, DW + P - 1] = wv - wv_hi
    wvd0 = np.zeros((P, DW), np.float32)
    shared = {"kT": kT, "V": Vm, "ind": ind, "mask": mask, "wvd0": wvd0,
              "wvdb": wvdb}
    in_maps = []
    for c in range(N_CORES):
        qT = np.concatenate(
            [queries[order[g], c * GROUP_ROWS:(c + 1) * GROUP_ROWS, :].T
             for g in range(B)], axis=1)
        bl = blob.copy()
        bl[:, 0:P] = qT
        m = dict(shared)
        m["blob"] = bl
        in_maps.append(m)

    if SIMULATE:
        from concourse.bass_interp import CoreSim
        outs = []
        for c in range(N_CORES):
            sim = CoreSim(nc, trace=False)
            for name, v in in_maps[c].items():
                sim.tensor(name)[:] = v
            sim.simulate(check_with_hw=False)
            outs.append(sim.tensor("out").copy())
    else:
        from concourse import bass_utils
        kw = {}
        if PROFILE:
            kw = {"trace": True}
        res = bass_utils.run_bass_kernel_spmd(nc, in_maps, list(range(N_CORES)),
                                              **kw)
        if PROFILE:
            LAST_EXEC_NS = res.exec_time_ns
            global LAST_RESULTS
            LAST_RESULTS = res
        outs = [res.results[c]["out"] for c in range(N_CORES)]

    out = np.zeros((B, Q, VS), np.float32)
    for c in range(N_CORES):
        for g in range(B):
            out[order[g], c * GROUP_ROWS:(c + 1) * GROUP_ROWS, :] = \
                outs[c][GROUP_ROWS * g: GROUP_ROWS * (g + 1), :]
    return out
